# revision 1
# baseline (speedup 1.0000x reference)
"""GCN (2-layer GraphConv + classifier) on 8 Trainium2 NeuronCores.

Sharding: nodes (and incident edges, by dst) across 8 cores; weights
replicated; per-layer AllGather of node features so every core gathers its
edges' source rows; segment-sum via one-hot M matmuls (fp8 M x bf16 data).

Key optimizations over the 1.6 ms baseline (measured on HW):
- The h AllGather is split into 2 chunks overlapped with phase-1 compute and
  phase-3 aggregation (two passes over src chunks; SBUF bf16 partials are
  re-seeded into PSUM via an identity matmul).
- Collective outputs are Local (not Shared) DRAM; gathers use SWDGE queue 0
  only (multi-queue round-robin measured slower in-kernel).
- z rows are written 256B-padded at the source and AllGathered padded, so the
  descriptor-bound z expand copy is gone.
- Group-level edge chunking with fixed per-tile slot windows (max over cores)
  lets 128-edge chunks span tile boundaries: ~11% fewer gathered rows.
- 4-tile gather groups with 4 rotating buffers (~55 KB of gather reads in
  flight) amortize the SWDGE per-call cost while hiding per-row latency.
"""
import os
import sys

sys.path.insert(0, "/opt/trn_rl_repo")

import numpy as np
import ml_dtypes

import concourse.bacc as bacc
import concourse.bass as bass
import concourse.mybir as mybir
import concourse.tile as tile
from concourse import library_config
from concourse.masks import make_identity

NCORES = 8
P = 128
N_NODES = 50000
N_EDGES = 400000
NP_PAD = 50176            # 8 * 6272
R = NP_PAD // NCORES      # 6272 rows per core
RT = R // P               # 49 row tiles per core
NS = 2                    # src chunks (pipeline stages for the AllGather)
CT0 = 25                  # tiles in chunk 0
CT1 = RT - CT0            # 24 tiles in chunk 1
CR = (CT0 * P, CT1 * P)   # rows per chunk per core (3200, 3072)
REG = (NCORES * CR[0], NCORES * CR[1])   # region sizes (25600, 24576) < 32768
IN_F = 1433
KP = 1536                 # padded contraction (12 * 128)
KC = KP // P              # 12 k-chunks
HID = 384
N_CLS = 7
ZC = 128                  # padded z row width (bf16 -> 256B rows for dma_gather)
GROUP_TILES = 4           # dst tiles per gather-call group
NQ = 1                    # SWDGE queues (4-queue round-robin measured slower in-kernel)

bf16 = ml_dtypes.bfloat16
fp8 = ml_dtypes.float8_e4m3


def _table_row(g):
    """Map padded global node id -> row in the chunked h table (vectorized)."""
    c = g // R
    r = g % R
    s = (r >= CR[0]).astype(np.int64)
    base = s * REG[0]
    cs = np.where(s == 0, CR[0], CR[1])
    rs = r - s * CR[0]
    return s, base + c * cs + rs


def _build_edge_plan(edge_src, edge_dst):
    """Group-level chunking: per (group, s) the tiles get fixed slot windows
    (max-over-cores edge counts, uniform so one NEFF fits all cores); 128-edge
    chunks span tile boundaries, with one matmul per (chunk, tile) overlap."""
    src = edge_src.astype(np.int64)
    dst = edge_dst.astype(np.int64)
    core = dst // R
    t_all = (dst % R) // P
    p_all = dst % P
    s_all, trow = _table_row(src)
    src_rel = trow - s_all * REG[0]   # region-relative row (< 32768)

    counts = np.zeros((NCORES, RT, NS), np.int64)
    np.add.at(counts, (core, t_all, s_all), 1)
    emax = np.maximum(counts.max(axis=0), 1)   # [RT, NS]

    # per-core edge arrays sorted by (core, tile, s, src)
    order = np.lexsort((src_rel, s_all, t_all, core))
    srt_core = core[order]
    srt_t = t_all[order]
    srt_s = s_all[order]
    srt_src = src_rel[order]
    srt_p = p_all[order]
    core_starts = np.searchsorted(srt_core, np.arange(NCORES + 1))
    # position of each edge within its (core, tile, s) run
    key_full = (srt_core * RT + srt_t) * NS + srt_s
    new_run = np.concatenate([[True], key_full[1:] != key_full[:-1]])
    run_start = np.nonzero(new_run)[0][np.cumsum(new_run) - 1]
    pos_in_run = np.arange(len(key_full)) - run_start

    n_groups = (RT + GROUP_TILES - 1) // GROUP_TILES
    groups = []          # (tiles, nch=[s0,s1], cbase, mmbase, mm_per_tile[s][t])
    chunk_base = 0
    mm_base = 0
    slot0 = np.zeros((RT, NS), np.int64)   # global slot of tile's window start
    mm_map = {}          # (global chunk, tile) -> mm index
    for g in range(n_groups):
        tiles = list(range(g * GROUP_TILES, min((g + 1) * GROUP_TILES, RT)))
        nch = []
        mm_per_tile = []
        cb = chunk_base
        mb = mm_base
        for s in range(NS):
            cbs = chunk_base
            ofs = 0
            mms = {}
            for t in tiles:
                slot0[t, s] = cbs * P + ofs
                c0 = ofs // P
                c1 = (ofs + emax[t, s] - 1) // P
                mms[t] = list(range(c0, c1 + 1))
                for ci in mms[t]:
                    mm_map[(cbs + ci, t)] = mm_base
                    mm_base += 1
                ofs += emax[t, s]
            n = (ofs + P - 1) // P
            nch.append(n)
            mm_per_tile.append(mms)
            chunk_base += n
        groups.append((tiles, nch, cb, mb, mm_per_tile))
    c_tot = chunk_base
    mm_tot = mm_base

    M_all = np.zeros((NCORES, mm_tot, P, P), fp8)
    idx_all = np.zeros((NCORES, c_tot * P), np.int64)

    # scatter edges into their fixed slots
    gslot = slot0[srt_t, srt_s] + pos_in_run
    mmidx_table = np.full((c_tot, RT), -1, np.int64)
    for (cg, t), mi in mm_map.items():
        mmidx_table[cg, t] = mi
    mm_e = mmidx_table[gslot // P, srt_t]
    assert (mm_e >= 0).all()
    for c in range(NCORES):
        a, e = core_starts[c], core_starts[c + 1]
        idx_all[c][gslot[a:e]] = srt_src[a:e]
        M_all[c][mm_e[a:e], gslot[a:e] % P, srt_p[a:e]] = 1

    # M pre-swizzled for DMA: [P(edge k), mm_tot*P(dst cols)]
    M_sw = np.ascontiguousarray(M_all.transpose(0, 2, 1, 3)).reshape(NCORES, P, mm_tot * P)

    idx_wrapped = np.zeros((NCORES, P, c_tot * P // 16), np.int16)
    for c in range(NCORES):
        w = idx_all[c].astype(np.int16).reshape(-1, 16).T
        idx_wrapped[c] = np.tile(w, (8, 1))

    return dict(
        emax=emax, groups=groups, c_tot=c_tot, mm_tot=mm_tot,
        M_sw=M_sw, idx_wrapped=idx_wrapped,
    )


def _build_nc(plan, repeat=1, upto=5, m_fp8=True, nq=NQ, shared_h=False,
              shared_z=False):
    """upto: 1=phase1 only, 2=+h AllGathers, 3=+phase3, 5=full (timing aid).

    shared_h/shared_z: addr_space of the collective output tables. Shared
    (pair-HBM) halves collective write traffic but dma_gather reads from a
    Shared region measured ~6x slower than Local — default Local."""
    groups = plan["groups"]
    c_tot = plan["c_tot"]
    mm_tot = plan["mm_tot"]

    nc = bacc.Bacc("TRN2", target_bir_lowering=False, debug=False,
                   num_devices=NCORES, num_swdge_queues=nq)
    dt = mybir.dt
    m_dt = dt.float8e4 if m_fp8 else dt.bfloat16
    gq = iter(range(10 ** 9))  # gather counter for queue round-robin

    # ---- I/O ----
    xT = nc.dram_tensor("xT", [RT, P, KC * P], dt.bfloat16, kind="ExternalInput")
    w1 = nc.dram_tensor("w1", [P, KC * HID], dt.bfloat16, kind="ExternalInput")
    w2c = nc.dram_tensor("w2c", [P, 3 * 8], dt.bfloat16, kind="ExternalInput")
    b1t = nc.dram_tensor("b1t", [P, HID], dt.float32, kind="ExternalInput")
    bct = nc.dram_tensor("bct", [P, 8], dt.float32, kind="ExternalInput")
    inv_s_t = nc.dram_tensor("inv_s_t", [P, RT], dt.float32, kind="ExternalInput")
    inv_d_t = nc.dram_tensor("inv_d_t", [P, RT], dt.float32, kind="ExternalInput")
    m_all = nc.dram_tensor("m_all", [P, mm_tot * P], m_dt, kind="ExternalInput")
    idxs = nc.dram_tensor("idxs", [P, c_tot * P // 16], dt.int16, kind="ExternalInput")
    out = nc.dram_tensor("out", [P, RT * N_CLS], dt.float32, kind="ExternalOutput")

    # ---- internal DRAM ----
    h_c = nc.dram_tensor("h_c", [R, HID], dt.bfloat16)
    h_full = nc.dram_tensor("h_full", [NP_PAD, HID], dt.bfloat16,
                            addr_space="Shared" if shared_h else "Local")
    z_c = nc.dram_tensor("z_c", [R, ZC], dt.bfloat16)
    z_full = nc.dram_tensor("z_full", [NP_PAD, ZC], dt.bfloat16,
                            addr_space="Shared" if shared_z else "Local")

    rg = [list(range(NCORES))]
    # (local h_c slice, h_full region) per src chunk
    h_slices = [(h_c[0:CR[0], :], h_full[0:REG[0], :]),
                (h_c[CR[0]:R, :], h_full[REG[0]:NP_PAD, :])]
    z_slices = [(z_c[0:CR[0], :], z_full[0:REG[0], :]),
                (z_c[CR[0]:R, :], z_full[REG[0]:NP_PAD, :])]
    zf_regions = [z_full[0:REG[0], :], z_full[REG[0]:NP_PAD, :]]

    with tile.TileContext(nc) as tc:
        with (
            tc.tile_pool(name="const", bufs=1) as const,
            tc.tile_pool(name="xload", bufs=2) as xload,
            tc.tile_pool(name="hout", bufs=3) as hout,
            tc.tile_pool(name="gbuf", bufs=4) as gbuf,
            tc.tile_pool(name="mbuf", bufs=4) as mbuf,
            tc.tile_pool(name="work", bufs=4) as work,
            tc.tile_pool(name="keep", bufs=1) as keep,
            tc.tile_pool(name="psA", bufs=2, space="PSUM") as psA,
            tc.tile_pool(name="psB", bufs=2, space="PSUM") as psB,
        ):
            nc.gpsimd.load_library(library_config.mlp)

            w1_t = const.tile([P, KC * HID], dt.bfloat16)
            nc.sync.dma_start(out=w1_t[:], in_=w1[:])
            w2c_t = const.tile([P, 3 * 8], dt.bfloat16)
            nc.sync.dma_start(out=w2c_t[:], in_=w2c[:])
            b1_t = const.tile([P, HID], dt.float32)
            nc.sync.dma_start(out=b1_t[:], in_=b1t[:])
            bc_t = const.tile([P, 8], dt.float32)
            nc.sync.dma_start(out=bc_t[:], in_=bct[:])
            invs_t = const.tile([P, RT], dt.float32)
            nc.sync.dma_start(out=invs_t[:], in_=inv_s_t[:])
            invd_t = const.tile([P, RT], dt.float32)
            nc.sync.dma_start(out=invd_t[:], in_=inv_d_t[:])
            idx_t = const.tile([P, c_tot * P // 16], dt.int16)
            nc.sync.dma_start(out=idx_t[:], in_=idxs[:])
            ident = const.tile([P, P], dt.bfloat16)
            make_identity(nc, ident[:])

            h1T_t = keep.tile([P, 3 * R], dt.bfloat16)  # [k=128][kchunk][row]
            partA = keep.tile([P, RT * HID], dt.bfloat16)   # phase-3 s0 partials
            partZ = keep.tile([P, RT * 8], dt.bfloat16)     # phase-5 s0 partials

            for _rep in range(repeat):
                # ---- Phase 1: h = (x @ W1) * inv_s, AllGather per chunk ----
                for r in range(RT):
                    xt = xload.tile([P, KC * P], dt.bfloat16)
                    nc.sync.dma_start(out=xt[:], in_=xT[r])
                    ps = psA.tile([P, HID], dt.float32, space="PSUM")
                    for k in range(KC):
                        nc.tensor.matmul(
                            out=ps[:],
                            lhsT=xt[:, k * P:(k + 1) * P],
                            rhs=w1_t[:, k * HID:(k + 1) * HID],
                            start=(k == 0),
                            stop=(k == KC - 1),
                        )
                    ht = hout.tile([P, HID], dt.bfloat16)
                    nc.scalar.activation(
                        out=ht[:], in_=ps[:],
                        func=mybir.ActivationFunctionType.Copy,
                        scale=invs_t[:, r:r + 1],
                    )
                    nc.sync.dma_start(out=h_c[r * P:(r + 1) * P, :], in_=ht[:])
                    if r == CT0 - 1 and upto >= 2:
                        nc.gpsimd.collective_compute(
                            "AllGather", mybir.AluOpType.bypass, replica_groups=rg,
                            ins=[h_slices[0][0]], outs=[h_slices[0][1]],
                        )
                if upto == 2:
                    nc.gpsimd.collective_compute(
                        "AllGather", mybir.AluOpType.bypass, replica_groups=rg,
                        ins=[h_slices[1][0]], outs=[h_slices[1][1]],
                    )
                if upto < 3:
                    continue

                # ---- Phase 3: aggregation 1 in two passes over src chunks ----
                # Pass A: s=0 edges -> partials in SBUF (bf16). The chunk-1
                # AllGather is emitted after the first gather group so pass-A
                # gathers (which only need chunk 0) aren't queued behind it.
                for gi, (tiles, nch, cbase, mmbase, mpt) in enumerate(groups):
                    n0 = nch[0]
                    mm0 = sum(len(v) for v in mpt[0].values())
                    if gi == 3:
                        nc.gpsimd.collective_compute(
                            "AllGather", mybir.AluOpType.bypass, replica_groups=rg,
                            ins=[h_slices[1][0]], outs=[h_slices[1][1]],
                        )
                    g_t = gbuf.tile([P, n0, HID], dt.bfloat16, tag="g1")
                    nc.gpsimd.dma_gather(
                        out_ap=g_t[:], in_ap=h_slices[0][1],
                        idxs_ap=idx_t[:, cbase * 8:(cbase + n0) * 8],
                        num_idxs=n0 * P, num_idxs_reg=n0 * P, elem_size=HID,
                        single_packet=False, queue_num=next(gq) % nq,
                    )
                    m_t = mbuf.tile([P, mm0 * P], m_dt, tag="m1")
                    nc.sync.dma_start(
                        out=m_t[:], in_=m_all[:, mmbase * P:(mmbase + mm0) * P])
                    mmoff = 0
                    for t in tiles:
                        cis = mpt[0][t]
                        ps = psA.tile([P, HID], dt.float32, space="PSUM", tag="ps")
                        for j, ci in enumerate(cis):
                            nc.tensor.matmul(
                                out=ps[:], lhsT=m_t[:, (mmoff + j) * P:(mmoff + j + 1) * P],
                                rhs=g_t[:, ci, :],
                                start=(j == 0), stop=(j == len(cis) - 1),
                            )
                        mmoff += len(cis)
                        nc.vector.tensor_copy(
                            out=partA[:, t * HID:(t + 1) * HID], in_=ps[:])
                # Pass B: seed with partials, add s=1 edges, epilogue
                for (tiles, nch, cbase, mmbase, mpt) in groups:
                    n0, n1 = nch[0], nch[1]
                    mm0 = sum(len(v) for v in mpt[0].values())
                    mm1 = sum(len(v) for v in mpt[1].values())
                    g_t = gbuf.tile([P, n1, HID], dt.bfloat16, tag="g1")
                    nc.gpsimd.dma_gather(
                        out_ap=g_t[:], in_ap=h_slices[1][1],
                        idxs_ap=idx_t[:, (cbase + n0) * 8:(cbase + n0 + n1) * 8],
                        num_idxs=n1 * P, num_idxs_reg=n1 * P, elem_size=HID,
                        single_packet=False, queue_num=next(gq) % nq,
                    )
                    m_t = mbuf.tile([P, mm1 * P], m_dt, tag="m1")
                    nc.sync.dma_start(
                        out=m_t[:],
                        in_=m_all[:, (mmbase + mm0) * P:(mmbase + mm0 + mm1) * P])
                    mmoff = 0
                    for t in tiles:
                        cis = mpt[1][t]
                        ps = psA.tile([P, HID], dt.float32, space="PSUM", tag="ps")
                        nc.tensor.matmul(
                            out=ps[:], lhsT=ident[:],
                            rhs=partA[:, t * HID:(t + 1) * HID],
                            start=True, stop=False,
                        )
                        for j, ci in enumerate(cis):
                            nc.tensor.matmul(
                                out=ps[:], lhsT=m_t[:, (mmoff + j) * P:(mmoff + j + 1) * P],
                                rhs=g_t[:, ci, :],
                                start=False, stop=(j == len(cis) - 1),
                            )
                        mmoff += len(cis)
                        # h1 = relu(agg * inv_d + b1)
                        tmp = work.tile([P, HID], dt.float32, tag="tmp1")
                        nc.vector.scalar_tensor_tensor(
                            out=tmp[:], in0=ps[:], scalar=invd_t[:, t:t + 1],
                            in1=b1_t[:],
                            op0=mybir.AluOpType.mult, op1=mybir.AluOpType.add,
                        )
                        h1t = work.tile([P, HID], dt.bfloat16, tag="h1t")
                        nc.vector.tensor_scalar_max(out=h1t[:], in0=tmp[:], scalar1=0.0)
                        for k in range(3):
                            pst = psB.tile([P, P], dt.bfloat16, space="PSUM", tag="pst")
                            nc.tensor.transpose(
                                out=pst[:], in_=h1t[:, k * P:(k + 1) * P],
                                identity=ident[:],
                            )
                            nc.vector.tensor_copy(
                                out=h1T_t[:, k * R + t * P: k * R + (t + 1) * P],
                                in_=pst[:],
                            )
                        psz = psB.tile([P, 8], dt.float32, space="PSUM", tag="psz")
                        for k in range(3):
                            nc.tensor.matmul(
                                out=psz[:],
                                lhsT=h1T_t[:, k * R + t * P: k * R + (t + 1) * P],
                                rhs=w2c_t[:, k * 8:(k + 1) * 8],
                                start=(k == 0), stop=(k == 2),
                            )
                        zt = work.tile([P, ZC], dt.bfloat16, tag="zt")
                        nc.scalar.activation(
                            out=zt[:, 0:8], in_=psz[:],
                            func=mybir.ActivationFunctionType.Copy,
                            scale=invs_t[:, t:t + 1],
                        )
                        nc.sync.dma_start(out=z_c[t * P:(t + 1) * P, :], in_=zt[:])
                if upto < 5:
                    continue

                # ---- Phase 5: aggregation 2 (7-wide), two passes ----
                # z chunk-0 collective: emitted after all pass-B gathers so it
                # doesn't block them at the pool queue head; it only waits for
                # z tiles 0..CT0-1 (mid pass B).
                nc.gpsimd.collective_compute(
                    "AllGather", mybir.AluOpType.bypass, replica_groups=rg,
                    ins=[z_slices[0][0]], outs=[z_slices[0][1]],
                )
                out_t = const.tile([P, RT * N_CLS], dt.float32)
                # Pass A: s=0 z edges -> partials (waits z chunk-0 collective)
                for (tiles, nch, cbase, mmbase, mpt) in groups:
                    n0 = nch[0]
                    mm0 = sum(len(v) for v in mpt[0].values())
                    g_t = gbuf.tile([P, n0, ZC], dt.bfloat16, tag="g2")
                    nc.gpsimd.dma_gather(
                        out_ap=g_t[:], in_ap=zf_regions[0],
                        idxs_ap=idx_t[:, cbase * 8:(cbase + n0) * 8],
                        num_idxs=n0 * P, num_idxs_reg=n0 * P, elem_size=ZC,
                        single_packet=False, queue_num=next(gq) % nq,
                    )
                    m_t = mbuf.tile([P, mm0 * P], m_dt, tag="m2")
                    nc.sync.dma_start(
                        out=m_t[:], in_=m_all[:, mmbase * P:(mmbase + mm0) * P])
                    mmoff = 0
                    for t in tiles:
                        cis = mpt[0][t]
                        ps = psB.tile([P, 8], dt.float32, space="PSUM", tag="ps2")
                        for j, ci in enumerate(cis):
                            nc.tensor.matmul(
                                out=ps[:], lhsT=m_t[:, (mmoff + j) * P:(mmoff + j + 1) * P],
                                rhs=g_t[:, ci, 0:8],
                                start=(j == 0), stop=(j == len(cis) - 1),
                            )
                        mmoff += len(cis)
                        nc.vector.tensor_copy(
                            out=partZ[:, t * 8:(t + 1) * 8], in_=ps[:])
                # z chunk-1 collective + expand (emitted after pass-A gathers so
                # those can start as soon as chunk-0 z is ready)
                nc.gpsimd.collective_compute(
                    "AllGather", mybir.AluOpType.bypass, replica_groups=rg,
                    ins=[z_slices[1][0]], outs=[z_slices[1][1]],
                )
                # Pass B: seed + s=1 z edges + output epilogue
                for (tiles, nch, cbase, mmbase, mpt) in groups:
                    n0, n1 = nch[0], nch[1]
                    mm0 = sum(len(v) for v in mpt[0].values())
                    mm1 = sum(len(v) for v in mpt[1].values())
                    g_t = gbuf.tile([P, n1, ZC], dt.bfloat16, tag="g2")
                    nc.gpsimd.dma_gather(
                        out_ap=g_t[:], in_ap=zf_regions[1],
                        idxs_ap=idx_t[:, (cbase + n0) * 8:(cbase + n0 + n1) * 8],
                        num_idxs=n1 * P, num_idxs_reg=n1 * P, elem_size=ZC,
                        single_packet=False, queue_num=next(gq) % nq,
                    )
                    m_t = mbuf.tile([P, mm1 * P], m_dt, tag="m2")
                    nc.sync.dma_start(
                        out=m_t[:],
                        in_=m_all[:, (mmbase + mm0) * P:(mmbase + mm0 + mm1) * P])
                    mmoff = 0
                    for t in tiles:
                        cis = mpt[1][t]
                        ps = psB.tile([P, 8], dt.float32, space="PSUM", tag="ps2")
                        nc.tensor.matmul(
                            out=ps[:], lhsT=ident[:],
                            rhs=partZ[:, t * 8:(t + 1) * 8],
                            start=True, stop=False,
                        )
                        for j, ci in enumerate(cis):
                            nc.tensor.matmul(
                                out=ps[:], lhsT=m_t[:, (mmoff + j) * P:(mmoff + j + 1) * P],
                                rhs=g_t[:, ci, 0:8],
                                start=False, stop=(j == len(cis) - 1),
                            )
                        mmoff += len(cis)
                        nc.vector.scalar_tensor_tensor(
                            out=out_t[:, t * N_CLS:(t + 1) * N_CLS],
                            in0=ps[:, 0:N_CLS], scalar=invd_t[:, t:t + 1],
                            in1=bc_t[:, 0:N_CLS],
                            op0=mybir.AluOpType.mult, op1=mybir.AluOpType.add,
                        )
                nc.sync.dma_start(out=out[:], in_=out_t[:])

    nc.compile()
    return nc


def _prepare(features, edge_src, edge_dst, W1, b1, W2, b2, Wc, bc):
    deg_out = np.bincount(edge_src, minlength=N_NODES).astype(np.float32)
    deg_in = np.bincount(edge_dst, minlength=N_NODES).astype(np.float32)
    inv_s = 1.0 / np.sqrt(np.maximum(deg_out, 1.0))
    inv_d = 1.0 / np.sqrt(np.maximum(deg_in, 1.0))
    inv_s = np.concatenate([inv_s, np.ones(NP_PAD - N_NODES, np.float32)])
    inv_d = np.concatenate([inv_d, np.ones(NP_PAD - N_NODES, np.float32)])

    plan = _build_edge_plan(edge_src, edge_dst)

    W1p = np.zeros((KP, HID), np.float32)
    W1p[:IN_F] = W1
    W1p = W1p.astype(bf16)
    w1_sw = np.concatenate([W1p[k * P:(k + 1) * P] for k in range(KC)], axis=1)
    W2c = (W2.astype(np.float32) @ Wc.astype(np.float32))
    W2cp = np.zeros((HID, 8), np.float32)
    W2cp[:, :N_CLS] = W2c
    W2cp16 = W2cp.astype(bf16)
    w2c_sw = np.concatenate([W2cp16[k * P:(k + 1) * P] for k in range(3)], axis=1)
    bcp = (b2.astype(np.float32) @ Wc.astype(np.float32) + bc).astype(np.float32)
    b1_full = np.tile(b1[None, :].astype(np.float32), (P, 1))
    bc_full = np.zeros((P, 8), np.float32)
    bc_full[:, :N_CLS] = bcp[None, :]

    xpad = np.zeros((NP_PAD, KP), bf16)
    xpad[:N_NODES, :IN_F] = features.astype(bf16)

    in_maps = []
    for c in range(NCORES):
        xt = np.ascontiguousarray(
            xpad[c * R:(c + 1) * R].reshape(RT, P, KC, P).transpose(0, 3, 2, 1)
        ).reshape(RT, P, KC * P)
        inv_s_tile = np.ascontiguousarray(inv_s[c * R:(c + 1) * R].reshape(RT, P).T)
        inv_d_tile = np.ascontiguousarray(inv_d[c * R:(c + 1) * R].reshape(RT, P).T)
        in_maps.append({
            "xT": xt,
            "w1": w1_sw,
            "w2c": w2c_sw,
            "b1t": b1_full,
            "bct": bc_full,
            "inv_s_t": inv_s_tile,
            "inv_d_t": inv_d_tile,
            "m_all": plan["M_sw"][c],
            "idxs": plan["idx_wrapped"][c],
        })
    return plan, in_maps


def kernel(features, edge_src, edge_dst, W1, b1, W2, b2, Wc, bc):
    features = np.asarray(features, np.float32)
    edge_src = np.asarray(edge_src)
    edge_dst = np.asarray(edge_dst)
    plan, in_maps = _prepare(features, edge_src, edge_dst,
                             np.asarray(W1, np.float32), np.asarray(b1, np.float32),
                             np.asarray(W2, np.float32), np.asarray(b2, np.float32),
                             np.asarray(Wc, np.float32), np.asarray(bc, np.float32))
    nc = _build_nc(plan)

    from concourse.bass_utils import run_bass_kernel_spmd
    res = run_bass_kernel_spmd(nc, in_maps, core_ids=list(range(NCORES)))

    out = np.zeros((NP_PAD, N_CLS), np.float32)
    for c in range(NCORES):
        buf = res.results[c]["out"]
        out[c * R:(c + 1) * R] = buf.reshape(P, RT, N_CLS).transpose(1, 0, 2).reshape(R, N_CLS)
    return out[:N_NODES]

